# revision 63
# baseline (speedup 1.0000x reference)
"""Multi-head attention (B=2, N=4096, C=512, H=8, d=64) on 8 Trainium2 NeuronCores.

Sharding: core c handles batch b = c//4 and heads {2*(c%4), 2*(c%4)+1}.
Each core computes its 2 heads' attention plus a partial output projection
(contraction over its 128 rows of W_proj); the host gather sums the 4
partials per batch (bias is added on the p==0 core of each batch).

On-device dataflow (transposed-scores formulation, no on-chip transposes):
  qT/kT [128=2*64 d-dims, 4096]  = W.T @ x.T      (x.T supplied by host)
  v_aug [128 n-chunk, 32*(65+65)] = x @ Wv with a ones column per head
  S^T[kidx, q] = kT.T_chunk @ qT  (two heads ride row-groups 0-1 / 2-3
                                   of the PE array concurrently, K=64 each)
  E = exp(S^T / 8)                (ScalarE, scale folded into the LUT affine)
  [out_unnorm^T; den] = v_aug.T @ E   (ones column makes row 64 the softmax
                                       denominator -- no extra pass)
  out^T = out_unnorm^T * (1/den)  (reciprocal + K=1 broadcast matmul)
  partial = out^T.T @ W_proj_slice + bias   (per-head K=64 contractions)
"""

import sys
import types

for _p in ("/opt/trn_rl_repo",):
    if _p not in sys.path:
        sys.path.insert(0, _p)

import numpy as np
import ml_dtypes
from contextlib import ExitStack

# antenv.axon_hooks shim: lets run_bass_kernel_spmd find the NTFF profiling
# hook when BASS_TRACE=1 (the agent image's antenv lacks this module).
import antenv  # noqa: F401

if "antenv.axon_hooks" not in sys.modules:
    _m = types.ModuleType("antenv.axon_hooks")
    _m._hook = None

    def _set_hook(h):
        _m._hook = h

    def _get_hook():
        return _m._hook

    _m.set_axon_ntff_profile_hook = _set_hook
    _m.get_axon_ntff_profile_hook = _get_hook
    sys.modules["antenv.axon_hooks"] = _m
    try:
        from trn_agent_boot.trn_boot import _ntff_profile_via_ctypes

        hook = _ntff_profile_via_ctypes("/opt/axon/libaxon_pjrt.so")
        if hook is not None:
            _set_hook(hook)
    except Exception:
        pass

import concourse.bass as bass  # noqa: E402
import concourse.tile as tile  # noqa: E402
from concourse.tile import add_dep_helper  # noqa: E402
from concourse import mybir, bacc  # noqa: E402
from concourse import bass_utils  # noqa: E402
from concourse import dve_ops as _dvo  # noqa: E402
from concourse.dve_spec import (  # noqa: E402
    lower as _dve_lower, Spec as _Spec, Src0 as _Src0,
    C0 as _C0, C1 as _C1, C2 as _C2, _has_src1,
)
from concourse.dve_uop import DveOpSpec as _DveOpSpec  # noqa: E402

# No bucket storage in this container; artifacts stay local.
bass_utils.upload_artifacts = lambda tmpdir: f"local://{tmpdir}"

B, N, C = 2, 4096, 512
H, D = 8, 64
N_CORES = 8
SCALE = D ** -0.5

# DVE-offloaded exp (Schraudolph in bf16-bit domain): a custom 3-stage
# DVE op computes round(s*EXP_A + EXP_B) via the fp32 magic-add trick
# ((x*A + B+M) - M forces round-to-integer at the +M add); the integral
# fp32 result is the bf16 BIT PATTERN of exp(s*SCALE)*(1+eps),
# |eps| <~ 4% mean-zero (cancels in the softmax ratio), and is written
# through a uint16-bitcast AP straight into the bf16 est tile.
EXP_G = 512           # q-columns per step exp'd on DVE instead of ScalarE
EXP_A = SCALE * 1.4426950408889634 * 128.0
EXP_SIGMA = 0.056
EXP_MAGIC = 12582912.0  # 1.5 * 2^23
EXP_B = 128.0 * (127.0 - EXP_SIGMA) + EXP_MAGIC

_exp_spec = _Spec(
    body=(_Src0 * _C0 + _C1) - _C2,
    reference=lambda in0, in1, s0, s1, imm2: (
        (in0.astype(np.float32) * np.float32(s0) + np.float32(s1)).astype(
            np.float32
        )
        - np.float32(imm2)
    ),
)
if "EXP_BF16_BITS_ANT" not in _dvo._SUB_OPCODE_FOR_NAME:
    _row = _dvo._CUSTOM_DVE_ROW_BASE + len(_dvo.OPS)
    _shas = {}
    for _ver in ("v3", "v4"):
        _s = _DveOpSpec(
            name="EXP_BF16_BITS_ANT", opcode=_row,
            uops=_dve_lower(_exp_spec, ver=_ver), rd1_en=_has_src1(_exp_spec),
        )
        _shas[_ver] = _s.sha(_ver)
    EXP_OP = _dvo.DveOp(
        "EXP_BF16_BITS_ANT", _exp_spec, subdim=False, uops_sha=_shas
    )
    _dvo.OPS.append(EXP_OP)
    _dvo.CUSTOM_DVE_SPECS[EXP_OP.name] = _exp_spec
    _dvo._SUB_OPCODE_FOR_NAME[EXP_OP.name] = _row
else:
    EXP_OP = next(o for o in _dvo.OPS if o.name == "EXP_BF16_BITS_ANT")

BF16 = mybir.dt.bfloat16
F32 = mybir.dt.float32
AF = mybir.ActivationFunctionType
BFNP = ml_dtypes.bfloat16

NI = N // 128   # 32 kidx / n chunks
NJ = N // 1024  # 4 q blocks
VW = 2 * (D + 1)  # 130: per-n-chunk vaug block (2 heads x (64 v + 1 ones))


def build_nc():
    nc = bacc.Bacc("TRN2", target_bir_lowering=False, debug=False)

    xt = nc.dram_tensor("xt", [4, 128, N], BF16, kind="ExternalInput").ap()
    wq = nc.dram_tensor("wq", [128, 512], BF16, kind="ExternalInput").ap()
    wk = nc.dram_tensor("wk", [128, 512], BF16, kind="ExternalInput").ap()
    wv = nc.dram_tensor("wv", [128, 512], BF16, kind="ExternalInput").ap()
    # wp rows 0-63 = W_proj rows of head 0, rows 64-127 = head 1 (the two
    # projection contractions ride PE row-groups 0-1 / 2-3 concurrently)
    wp = nc.dram_tensor("wp", [128, 512], BF16, kind="ExternalInput").ap()
    # bias pre-replicated across partitions; fp32 (added via affine_then_add)
    bias = nc.dram_tensor("bias", [128, 512], F32, kind="ExternalInput").ap()
    out = nc.dram_tensor("out", [N, C], F32, kind="ExternalOutput").ap()
    # DRAM bounce buffer for transposing den rows to partition-major
    # (SBUF source APs cannot step partitions by elements; DRAM is linear)
    dend = nc.dram_tensor("dend", [2, 512], BF16, kind="Internal").ap()

    with tile.TileContext(nc) as tc:
        with ExitStack() as ctx:
            const = ctx.enter_context(tc.tile_pool(name="const", bufs=1))
            sb = ctx.enter_context(tc.tile_pool(name="sb", bufs=1))
            expp = ctx.enter_context(tc.tile_pool(name="expp", bufs=4))
            invp = ctx.enter_context(tc.tile_pool(name="invp", bufs=2))
            outp = ctx.enter_context(tc.tile_pool(name="outp", bufs=3))

            wrmw = const.tile([128, 128], BF16)
            nc.gpsimd.memset(wrmw[:], 0.5)
            twq = const.tile([128, 512], BF16)
            nc.gpsimd.dma_start(twq[:], wq[:])
            twk = const.tile([128, 512], BF16)
            nc.gpsimd.dma_start(twk[:], wk[:])
            twv = const.tile([128, 512], BF16)
            nc.gpsimd.dma_start(twv[:], wv[:])
            # twp/biasr are not needed until the first block tail (~step 48);
            # their DMAs are emitted right before the main loop so they don't
            # compete with the x.T load on the critical path to score 0.
            twp = const.tile([128, 512], BF16)
            biasr = const.tile([128, 512], F32)

            qT = sb.tile([128, N], BF16)
            kT = sb.tile([128, N], BF16)
            vaug = sb.tile([128, NI * VW], BF16)
            nc.gpsimd.memset(vaug[:], 1.0)
            # unnormalized out^T, both heads stacked: h0 rows 0-63, h1 rows
            # 64-127 (normalization by 1/den is folded into the projection
            # evacuation via per-partition affine_then_add scales)
            outT = sb.tile([128, N], BF16)

            xtp = ctx.enter_context(tc.tile_pool(name="xtp", bufs=1))
            psS0 = ctx.enter_context(tc.tile_pool(name="psS0", bufs=2, space="PSUM"))
            psS1 = ctx.enter_context(tc.tile_pool(name="psS1", bufs=2, space="PSUM"))
            psAV = ctx.enter_context(tc.tile_pool(name="psAV", bufs=1, space="PSUM"))
            psT = ctx.enter_context(tc.tile_pool(name="psT", bufs=2, space="PSUM"))

            # ---- stage A: QKV projections ------------------------------
            # Emitted as deadline-scheduled tasks threaded into the first two
            # j-blocks' i-loops (the PE queue is strict FIFO; anything emitted
            # before the first score matmul delays the first exp).
            xts = []
            for k in range(4):
                t = xtp.tile([128, N], BF16, tag=f"xt{k}", name=f"xt{k}")
                xts.append(t)
            for col in range(8):
                for k in range(4):
                    cs = bass.ts(col, N // 8)
                    nc.sync.dma_start(xts[k][:, cs], xt[k][:, cs])

            # PE warmup: the HAM clock gate releases (1.2 -> 2.4 GHz) only
            # after ~3.4us of sustained PE activity; burn that window on
            # dummy matmuls (on a memset tile, so they start immediately)
            # while the x.T DMA lands, so stage A runs warm.
            warm = psT.tile([128, 64], F32, tag="t", name="warm")
            for _ in range(48):
                nc.tensor.matmul(
                    warm[:], wrmw[:, 0:128], wrmw[:, 0:64],
                    start=True, stop=True,
                )

            def emit_qk(j8, which):
                s_ = bass.ts(j8, 512)
                w, dst = (twq, qT) if which == "q" else (twk, kT)
                ps = psT.tile([128, 512], F32, tag="t", name="psqk")
                for k in range(4):
                    nc.tensor.matmul(
                        ps[:], w[:, bass.ts(k, 128)], xts[k][:, s_],
                        start=(k == 0), stop=(k == 3),
                    )
                nc.scalar.copy(dst[:, s_], ps[:])

            def emit_v(jj):
                ps = psT.tile([128, 128], F32, tag="t", name="psv")
                for k in range(4):
                    nc.tensor.matmul(
                        ps[:], xts[k][:, bass.ts(jj, 128)], twv[:, bass.ts(k, 128)],
                        start=(k == 0), stop=(k == 3),
                    )
                dst = vaug[:, jj * VW : (jj + 1) * VW].rearrange(
                    "p (h c) -> p h c", h=2
                )[:, :, 0:D]
                src = ps[:].rearrange("p (h c) -> p h c", h=2)
                nc.vector.tensor_copy(dst, src)

            # (deadline in global i-steps, emitter) — qk k-chunk c feeds
            # scores at step 4c; v chunk jj feeds the AV matmul at step jj;
            # qk q-chunk j8 feeds block j8 (step 32*j8).
            stage_a_tasks = []
            for c in range(1, 8):
                stage_a_tasks.append((4 * c - 4, lambda c=c: emit_qk(c, "k")))
            for jj in range(1, NI):
                stage_a_tasks.append((jj - 2, lambda jj=jj: emit_v(jj)))
            for j8 in range(1, 8):
                stage_a_tasks.append((32 * j8 - 6, lambda j8=j8: emit_qk(j8, "q")))
            stage_a_tasks.sort(key=lambda t: t[0])
            stage_a_tasks = list(stage_a_tasks)

            # prologue: what step 0 needs
            emit_qk(0, "q")
            emit_qk(0, "k")
            emit_v(0)
            nc.gpsimd.dma_start(twp[:], wp[:])
            nc.gpsimd.dma_start(biasr[:], bias[:])

            # ---- stage B: scores^T -> exp -> AV (+den) ------------------
            # ---- stage C: paired projection, normalize folded in --------
            # Block tail: evacuate psAV to bf16 outT (h1 hops to partitions
            # 64-127 via SBUF DMA), transpose the den rows to a [128, 8]
            # partition-major tile via the DMA xbar, one fast reciprocal,
            # then per-jj: two row-group-CONCURRENT projection matmuls into
            # separate PSUM tiles and two affine_then_add evacuations that
            # apply 1/den (per-partition scale) and the bias.
            def emit_recip(denT16):
                denT32 = invp.tile([128, 8], F32, tag="d32", name="d32")
                nc.vector.tensor_copy(denT32[:], denT16[:])
                invT = invp.tile([128, 8], F32, tag="invT", name="invT")
                nc.vector.reciprocal_approx_fast(invT[:], denT32[:])
                return invT

            def emit_proj(j, k, invT, after=None):
                # Returns a closure finishing the second half (ATA #2 + store)
                # so the caller can defer it one step: two back-to-back 690ns
                # DVE ATAs would head-of-line-block the est32 op the next AV
                # matmul needs.
                jj = j * 4 + k
                s = bass.ts(jj, 128)
                pp0 = psT.tile([128, 512], F32, tag="t", name="pp0")
                mi = nc.tensor.matmul(
                    pp0[:], outT[0:64, s], twp[0:64, :], start=True, stop=True
                )
                if after is not None:
                    add_dep_helper(mi.ins, after.ins, sync=False,
                                   reason="tail behind scores")
                pp1 = psT.tile([128, 512], F32, tag="t", name="pp1")
                nc.tensor.matmul(
                    pp1[:], outT[64:128, s], twp[64:128, :],
                    start=True, stop=True,
                )
                ota = outp.tile([128, 512], F32, tag="oa", name="ota")
                nc.vector.affine_then_add(
                    ota[:], pp0[:], biasr[:], invT[:, k : k + 1], 0.0
                )

                def fin2():
                    ot = outp.tile([128, 512], F32, tag="o", name="ot")
                    nc.vector.affine_then_add(
                        ot[:], pp1[:], ota[:], invT[:, 4 + k : 4 + k + 1], 0.0
                    )
                    nc.sync.dma_start(out[s, :], ot[:])

                return fin2

            # Flat software pipeline over all 256 i-steps. AV matmuls are
            # emitted one step behind their scores/exp so the PE queue always
            # holds independent score work when an AV has to wait (block
            # boundary: the new accumulator bank frees only after the old
            # one's DVE evacuation).
            prev = None   # pending proj tail of the finished block
            # AV emission delayed TWO steps behind its scores: the exp of a
            # step is serialized (ScalarE half then DVE half -- same est tile,
            # so the framework orders the writers) at ~1.35us latency; depth-2
            # pipelining keeps that latency entirely off the PE critical path.
            pendq = []
            avs = None
            deferred = []  # per-block tail pieces drained one per step
            NT = 8 * NI
            PEND_DEPTH = 2
            def emit_head_tail(h, p_avs, p_qs, denT16):
                tmp65 = invp.tile([65, 512], BF16, tag=f"tmp{h}",
                                  name=f"tmp{h}")
                # evacuation casts run on ScalarE: the DVE (custom-op exp) is
                # the fuller engine once EXP_G=512
                nc.scalar.copy(tmp65[:], p_avs[h][:])
                nc.sync.dma_start(outT[h * 64 : h * 64 + 64, p_qs],
                                  tmp65[0:64, :])
                nc.gpsimd.dma_start(dend[h : h + 1, :], tmp65[64:65, :])
                nc.gpsimd.dma_start(
                    denT16[:, 4 * h : 4 * h + 4],
                    dend[h : h + 1, :].rearrange("one (k p) -> p (one k)",
                                                 p=128),
                )

            for gs in range(NT + PEND_DEPTH):
                j, i = divmod(gs, NI)
                # ---- AV matmuls of step gs-2 FIRST: they are ready (their
                # exp finished mid-previous-step) and must not queue behind
                # this step's scores, which wait on exp(gs-2) via the psS
                # buffer rotation.
                if len(pendq) == PEND_DEPTH or (gs >= NT and pendq):
                    p_avs, p_est, p_est_v, p_start, p_stop, p_i, p_qs, p_j = \
                        pendq.pop(0)
                    va0 = vaug[:, p_i * VW + 0 * 65 : p_i * VW + 1 * 65]
                    va1 = vaug[:, p_i * VW + 1 * 65 : p_i * VW + 2 * 65]
                    nc.tensor.matmul(
                        p_avs[0][:], va0, p_est[:],
                        start=p_start, stop=p_stop,
                    )
                    nc.tensor.matmul(
                        p_avs[1][:], va1, p_est_v[:],
                        start=p_start, stop=p_stop,
                    )
                    if p_stop:
                        # psAV evacuation + den transposes: deferred to after
                        # this step's exp emission (same gs -- still ahead of
                        # the next block's first AV write at gs+1) so the DVE
                        # FIFO never front-runs exp with the 2x658ns casts.
                        denT16 = invp.tile([128, 8], BF16, tag="dT",
                                           name="dT")
                        st = {"j": p_j}
                        for h in range(2):
                            deferred.append(
                                (gs, lambda h=h, a=p_avs, q=p_qs, d=denT16:
                                 emit_head_tail(h, a, q, d))
                            )

                        def _fin(st=st, d=denT16):
                            st["invT"] = emit_recip(d)

                        deferred.append((gs + 8, _fin))
                        prev = st
                if gs < NT:
                    if i == 0:
                        avs = [
                            psAV.tile([65, 512], F32, tag=f"av{t}", name=f"av{t}")
                            for t in range(2)
                        ]
                    while stage_a_tasks and stage_a_tasks[0][0] <= gs + 3:
                        stage_a_tasks.pop(0)[1]()
                    if prev is not None and i >= 16 and i % 4 == 0:
                        fin2 = emit_proj(prev["j"], (i - 16) // 4,
                                         prev["invT"], after=last_sc)
                        deferred.append((gs + 1, fin2))
                    qs = bass.ts(j, 512)
                    ks = bass.ts(i, 128)
                    # per-head score tiles in per-head pools: head 0's exp
                    # runs on ScalarE, head 1's on the custom DVE op — each
                    # engine frees its own PSUM ring independently, and the
                    # est tiles stay fully decoupled (no WAW serialization).
                    pss0 = psS0.tile([128, 512], F32, tag="s0")
                    pss1 = psS1.tile([128, 512], F32, tag="s1")
                    nc.tensor.matmul(
                        pss0[:], kT[0:64, ks], qT[0:64, qs],
                        start=True, stop=True,
                    )
                    last_sc = nc.tensor.matmul(
                        pss1[:], kT[64:128, ks], qT[64:128, qs],
                        start=True, stop=True,
                    )
                    est_v = expp.tile([128, 512], BF16, tag="ev")
                    nc.vector._custom_dve(
                        EXP_OP,
                        out=est_v[:].bitcast(mybir.dt.uint16),
                        in0=pss1[:],
                        s0=EXP_A, s1=EXP_B, imm2=EXP_MAGIC,
                    )
                    est = expp.tile([128, 512], BF16, tag="e")
                    nc.scalar.activation(
                        est[:], pss0[:], AF.Exp, scale=SCALE,
                    )
                while deferred and (gs >= NT or deferred[0][0] <= gs):
                    deferred.pop(0)[1]()
                if gs < NT:
                    pendq.append((avs, est, est_v, i == 0, i == NI - 1, i,
                                  bass.ts(j, 512), j))
            # final block's tail (no next block hides it)
            lj = prev["j"]
            fins = [emit_proj(lj, k, prev["invT"]) for k in range(4)]
            for f in fins:
                f()

    nc.compile()
    return nc


def _pack_w(wslice):
    # [512, 128] -> SBUF image [128, 4*128] with C-chunk k at cols k*128..
    return np.ascontiguousarray(
        wslice.reshape(4, 128, 128).transpose(1, 0, 2).reshape(128, 512)
    ).astype(BFNP)


_NC_CACHE = None
LAST_RESULT = None


def kernel(x, W_qkv, W_proj, b_proj):
    global _NC_CACHE, LAST_RESULT
    x = np.asarray(x, dtype=np.float32)
    W_qkv = np.asarray(W_qkv, dtype=np.float32)
    W_proj = np.asarray(W_proj, dtype=np.float32)
    b_proj = np.asarray(b_proj, dtype=np.float32)

    if _NC_CACHE is None:
        _NC_CACHE = build_nc()
    nc = _NC_CACHE

    in_maps = []
    for c in range(N_CORES):
        b = c // 4
        h0 = 2 * (c % 4)
        xtb = np.ascontiguousarray(x[b].T).reshape(4, 128, N).astype(BFNP)
        wq = _pack_w(W_qkv[:, h0 * 64 : h0 * 64 + 128])
        wk = _pack_w(W_qkv[:, 512 + h0 * 64 : 512 + h0 * 64 + 128])
        wv = _pack_w(W_qkv[:, 1024 + h0 * 64 : 1024 + h0 * 64 + 128])
        wp = np.ascontiguousarray(
            W_proj[h0 * 64 : (h0 + 2) * 64, :]
        ).astype(BFNP)
        bias = (
            np.tile(b_proj[None, :], (128, 1)).astype(np.float32)
            if c % 4 == 0
            else np.zeros((128, 512), dtype=np.float32)
        )
        in_maps.append(
            {"xt": xtb, "wq": wq, "wk": wk, "wv": wv, "wp": wp, "bias": bias}
        )

    res = bass_utils.run_bass_kernel_spmd(
        nc, in_maps, core_ids=list(range(N_CORES))
    )
    LAST_RESULT = res

    out = np.zeros((B, N, C), dtype=np.float32)
    for c in range(N_CORES):
        out[c // 4] += res.results[c]["out"]
    return out



# revision 64
# speedup vs baseline: 1.0002x; 1.0002x over previous
"""Multi-head attention (B=2, N=4096, C=512, H=8, d=64) on 8 Trainium2 NeuronCores.

Sharding: core c handles batch b = c//4 and heads {2*(c%4), 2*(c%4)+1}.
Each core computes its 2 heads' attention plus a partial output projection
(contraction over its 128 rows of W_proj); the host gather sums the 4
partials per batch (bias is added on the p==0 core of each batch).

On-device dataflow (transposed-scores formulation, no on-chip transposes):
  qT/kT [128=2*64 d-dims, 4096]  = W.T @ x.T      (x.T supplied by host)
  v_aug [128 n-chunk, 32*(65+65)] = x @ Wv with a ones column per head
  S^T[kidx, q] = kT.T_chunk @ qT  (two heads ride row-groups 0-1 / 2-3
                                   of the PE array concurrently, K=64 each)
  E = exp(S^T / 8)                (ScalarE, scale folded into the LUT affine)
  [out_unnorm^T; den] = v_aug.T @ E   (ones column makes row 64 the softmax
                                       denominator -- no extra pass)
  out^T = out_unnorm^T * (1/den)  (reciprocal + K=1 broadcast matmul)
  partial = out^T.T @ W_proj_slice + bias   (per-head K=64 contractions)
"""

import sys
import types

for _p in ("/opt/trn_rl_repo",):
    if _p not in sys.path:
        sys.path.insert(0, _p)

import numpy as np
import ml_dtypes
from contextlib import ExitStack

# antenv.axon_hooks shim: lets run_bass_kernel_spmd find the NTFF profiling
# hook when BASS_TRACE=1 (the agent image's antenv lacks this module).
import antenv  # noqa: F401

if "antenv.axon_hooks" not in sys.modules:
    _m = types.ModuleType("antenv.axon_hooks")
    _m._hook = None

    def _set_hook(h):
        _m._hook = h

    def _get_hook():
        return _m._hook

    _m.set_axon_ntff_profile_hook = _set_hook
    _m.get_axon_ntff_profile_hook = _get_hook
    sys.modules["antenv.axon_hooks"] = _m
    try:
        from trn_agent_boot.trn_boot import _ntff_profile_via_ctypes

        hook = _ntff_profile_via_ctypes("/opt/axon/libaxon_pjrt.so")
        if hook is not None:
            _set_hook(hook)
    except Exception:
        pass

import concourse.bass as bass  # noqa: E402
import concourse.tile as tile  # noqa: E402
from concourse.tile import add_dep_helper  # noqa: E402
from concourse import mybir, bacc  # noqa: E402
from concourse import bass_utils  # noqa: E402
from concourse import dve_ops as _dvo  # noqa: E402
from concourse.dve_spec import (  # noqa: E402
    lower as _dve_lower, Spec as _Spec, Src0 as _Src0,
    C0 as _C0, C1 as _C1, C2 as _C2, _has_src1,
)
from concourse.dve_uop import DveOpSpec as _DveOpSpec  # noqa: E402

# No bucket storage in this container; artifacts stay local.
bass_utils.upload_artifacts = lambda tmpdir: f"local://{tmpdir}"

B, N, C = 2, 4096, 512
H, D = 8, 64
N_CORES = 8
SCALE = D ** -0.5

# DVE-offloaded exp (Schraudolph in bf16-bit domain): a custom 3-stage
# DVE op computes round(s*EXP_A + EXP_B) via the fp32 magic-add trick
# ((x*A + B+M) - M forces round-to-integer at the +M add); the integral
# fp32 result is the bf16 BIT PATTERN of exp(s*SCALE)*(1+eps),
# |eps| <~ 4% mean-zero (cancels in the softmax ratio), and is written
# through a uint16-bitcast AP straight into the bf16 est tile.
EXP_G = 512           # q-columns per step exp'd on DVE instead of ScalarE
EXP_A = SCALE * 1.4426950408889634 * 128.0
EXP_SIGMA = 0.056
EXP_MAGIC = 12582912.0  # 1.5 * 2^23
EXP_B = 128.0 * (127.0 - EXP_SIGMA) + EXP_MAGIC

_exp_spec = _Spec(
    body=(_Src0 * _C0 + _C1) - _C2,
    reference=lambda in0, in1, s0, s1, imm2: (
        (in0.astype(np.float32) * np.float32(s0) + np.float32(s1)).astype(
            np.float32
        )
        - np.float32(imm2)
    ),
)
if "EXP_BF16_BITS_ANT" not in _dvo._SUB_OPCODE_FOR_NAME:
    _row = _dvo._CUSTOM_DVE_ROW_BASE + len(_dvo.OPS)
    _shas = {}
    for _ver in ("v3", "v4"):
        _s = _DveOpSpec(
            name="EXP_BF16_BITS_ANT", opcode=_row,
            uops=_dve_lower(_exp_spec, ver=_ver), rd1_en=_has_src1(_exp_spec),
        )
        _shas[_ver] = _s.sha(_ver)
    EXP_OP = _dvo.DveOp(
        "EXP_BF16_BITS_ANT", _exp_spec, subdim=False, uops_sha=_shas
    )
    _dvo.OPS.append(EXP_OP)
    _dvo.CUSTOM_DVE_SPECS[EXP_OP.name] = _exp_spec
    _dvo._SUB_OPCODE_FOR_NAME[EXP_OP.name] = _row
else:
    EXP_OP = next(o for o in _dvo.OPS if o.name == "EXP_BF16_BITS_ANT")

BF16 = mybir.dt.bfloat16
F32 = mybir.dt.float32
AF = mybir.ActivationFunctionType
BFNP = ml_dtypes.bfloat16

NI = N // 128   # 32 kidx / n chunks
NJ = N // 1024  # 4 q blocks
VW = 2 * (D + 1)  # 130: per-n-chunk vaug block (2 heads x (64 v + 1 ones))


def build_nc():
    nc = bacc.Bacc("TRN2", target_bir_lowering=False, debug=False)

    xt = nc.dram_tensor("xt", [4, 128, N], BF16, kind="ExternalInput").ap()
    wq = nc.dram_tensor("wq", [128, 512], BF16, kind="ExternalInput").ap()
    wk = nc.dram_tensor("wk", [128, 512], BF16, kind="ExternalInput").ap()
    wv = nc.dram_tensor("wv", [128, 512], BF16, kind="ExternalInput").ap()
    # wp rows 0-63 = W_proj rows of head 0, rows 64-127 = head 1 (the two
    # projection contractions ride PE row-groups 0-1 / 2-3 concurrently)
    wp = nc.dram_tensor("wp", [128, 512], BF16, kind="ExternalInput").ap()
    # bias pre-replicated across partitions; fp32 (added via affine_then_add)
    bias = nc.dram_tensor("bias", [128, 512], F32, kind="ExternalInput").ap()
    out = nc.dram_tensor("out", [N, C], F32, kind="ExternalOutput").ap()
    # DRAM bounce buffer for transposing den rows to partition-major
    # (SBUF source APs cannot step partitions by elements; DRAM is linear)
    dend = nc.dram_tensor("dend", [2, 512], BF16, kind="Internal").ap()

    with tile.TileContext(nc) as tc:
        with ExitStack() as ctx:
            const = ctx.enter_context(tc.tile_pool(name="const", bufs=1))
            sb = ctx.enter_context(tc.tile_pool(name="sb", bufs=1))
            expp = ctx.enter_context(tc.tile_pool(name="expp", bufs=4))
            invp = ctx.enter_context(tc.tile_pool(name="invp", bufs=2))
            outp = ctx.enter_context(tc.tile_pool(name="outp", bufs=3))

            wrmw = const.tile([128, 128], BF16)
            nc.gpsimd.memset(wrmw[:], 0.5)
            twq = const.tile([128, 512], BF16)
            nc.gpsimd.dma_start(twq[:], wq[:])
            twk = const.tile([128, 512], BF16)
            nc.gpsimd.dma_start(twk[:], wk[:])
            twv = const.tile([128, 512], BF16)
            nc.gpsimd.dma_start(twv[:], wv[:])
            # twp/biasr are not needed until the first block tail (~step 48);
            # their DMAs are emitted right before the main loop so they don't
            # compete with the x.T load on the critical path to score 0.
            twp = const.tile([128, 512], BF16)
            biasr = const.tile([128, 512], F32)

            qT = sb.tile([128, N], BF16)
            kT = sb.tile([128, N], BF16)
            vaug = sb.tile([128, NI * VW], BF16)
            nc.gpsimd.memset(vaug[:], 1.0)
            # unnormalized out^T, both heads stacked: h0 rows 0-63, h1 rows
            # 64-127 (normalization by 1/den is folded into the projection
            # evacuation via per-partition affine_then_add scales)
            outT = sb.tile([128, N], BF16)

            xtp = ctx.enter_context(tc.tile_pool(name="xtp", bufs=1))
            psS0 = ctx.enter_context(tc.tile_pool(name="psS0", bufs=2, space="PSUM"))
            psS1 = ctx.enter_context(tc.tile_pool(name="psS1", bufs=2, space="PSUM"))
            psAV = ctx.enter_context(tc.tile_pool(name="psAV", bufs=1, space="PSUM"))
            psT = ctx.enter_context(tc.tile_pool(name="psT", bufs=2, space="PSUM"))

            # ---- stage A: QKV projections ------------------------------
            # Emitted as deadline-scheduled tasks threaded into the first two
            # j-blocks' i-loops (the PE queue is strict FIFO; anything emitted
            # before the first score matmul delays the first exp).
            xts = []
            for k in range(4):
                t = xtp.tile([128, N], BF16, tag=f"xt{k}", name=f"xt{k}")
                xts.append(t)
            for col in range(8):
                for k in range(4):
                    cs = bass.ts(col, N // 8)
                    nc.sync.dma_start(xts[k][:, cs], xt[k][:, cs])

            # PE warmup: the HAM clock gate releases (1.2 -> 2.4 GHz) only
            # after ~3.4us of sustained PE activity; burn that window on
            # dummy matmuls (on a memset tile, so they start immediately)
            # while the x.T DMA lands, so stage A runs warm.
            warm = psT.tile([128, 64], F32, tag="t", name="warm")
            for _ in range(48):
                nc.tensor.matmul(
                    warm[:], wrmw[:, 0:128], wrmw[:, 0:64],
                    start=True, stop=True,
                )

            def emit_qk(j8, which):
                s_ = bass.ts(j8, 512)
                w, dst = (twq, qT) if which == "q" else (twk, kT)
                ps = psT.tile([128, 512], F32, tag="t", name="psqk")
                for k in range(4):
                    nc.tensor.matmul(
                        ps[:], w[:, bass.ts(k, 128)], xts[k][:, s_],
                        start=(k == 0), stop=(k == 3),
                    )
                nc.vector.tensor_copy(dst[:, s_], ps[:])

            def emit_v(jj):
                ps = psT.tile([128, 128], F32, tag="t", name="psv")
                for k in range(4):
                    nc.tensor.matmul(
                        ps[:], xts[k][:, bass.ts(jj, 128)], twv[:, bass.ts(k, 128)],
                        start=(k == 0), stop=(k == 3),
                    )
                dst = vaug[:, jj * VW : (jj + 1) * VW].rearrange(
                    "p (h c) -> p h c", h=2
                )[:, :, 0:D]
                src = ps[:].rearrange("p (h c) -> p h c", h=2)
                nc.vector.tensor_copy(dst, src)

            # (deadline in global i-steps, emitter) — qk k-chunk c feeds
            # scores at step 4c; v chunk jj feeds the AV matmul at step jj;
            # qk q-chunk j8 feeds block j8 (step 32*j8).
            stage_a_tasks = []
            for c in range(1, 8):
                stage_a_tasks.append((4 * c - 4, lambda c=c: emit_qk(c, "k")))
            for jj in range(1, NI):
                stage_a_tasks.append((jj - 2, lambda jj=jj: emit_v(jj)))
            for j8 in range(1, 8):
                stage_a_tasks.append((32 * j8 - 6, lambda j8=j8: emit_qk(j8, "q")))
            stage_a_tasks.sort(key=lambda t: t[0])
            stage_a_tasks = list(stage_a_tasks)

            # prologue: what step 0 needs
            emit_qk(0, "q")
            emit_qk(0, "k")
            emit_v(0)
            nc.gpsimd.dma_start(twp[:], wp[:])
            nc.gpsimd.dma_start(biasr[:], bias[:])

            # ---- stage B: scores^T -> exp -> AV (+den) ------------------
            # ---- stage C: paired projection, normalize folded in --------
            # Block tail: evacuate psAV to bf16 outT (h1 hops to partitions
            # 64-127 via SBUF DMA), transpose the den rows to a [128, 8]
            # partition-major tile via the DMA xbar, one fast reciprocal,
            # then per-jj: two row-group-CONCURRENT projection matmuls into
            # separate PSUM tiles and two affine_then_add evacuations that
            # apply 1/den (per-partition scale) and the bias.
            def emit_recip(denT16):
                denT32 = invp.tile([128, 8], F32, tag="d32", name="d32")
                nc.vector.tensor_copy(denT32[:], denT16[:])
                invT = invp.tile([128, 8], F32, tag="invT", name="invT")
                nc.vector.reciprocal_approx_fast(invT[:], denT32[:])
                return invT

            def emit_proj(j, k, invT, after=None):
                # Returns a closure finishing the second half (ATA #2 + store)
                # so the caller can defer it one step: two back-to-back 690ns
                # DVE ATAs would head-of-line-block the est32 op the next AV
                # matmul needs.
                jj = j * 4 + k
                s = bass.ts(jj, 128)
                pp0 = psT.tile([128, 512], F32, tag="t", name="pp0")
                mi = nc.tensor.matmul(
                    pp0[:], outT[0:64, s], twp[0:64, :], start=True, stop=True
                )
                if after is not None:
                    add_dep_helper(mi.ins, after.ins, sync=False,
                                   reason="tail behind scores")
                pp1 = psT.tile([128, 512], F32, tag="t", name="pp1")
                nc.tensor.matmul(
                    pp1[:], outT[64:128, s], twp[64:128, :],
                    start=True, stop=True,
                )
                ota = outp.tile([128, 512], F32, tag="oa", name="ota")
                nc.vector.affine_then_add(
                    ota[:], pp0[:], biasr[:], invT[:, k : k + 1], 0.0
                )

                def fin2():
                    ot = outp.tile([128, 512], F32, tag="o", name="ot")
                    nc.vector.affine_then_add(
                        ot[:], pp1[:], ota[:], invT[:, 4 + k : 4 + k + 1], 0.0
                    )
                    nc.sync.dma_start(out[s, :], ot[:])

                return fin2

            # Flat software pipeline over all 256 i-steps. AV matmuls are
            # emitted one step behind their scores/exp so the PE queue always
            # holds independent score work when an AV has to wait (block
            # boundary: the new accumulator bank frees only after the old
            # one's DVE evacuation).
            prev = None   # pending proj tail of the finished block
            # AV emission delayed TWO steps behind its scores: the exp of a
            # step is serialized (ScalarE half then DVE half -- same est tile,
            # so the framework orders the writers) at ~1.35us latency; depth-2
            # pipelining keeps that latency entirely off the PE critical path.
            pendq = []
            avs = None
            deferred = []  # per-block tail pieces drained one per step
            NT = 8 * NI
            PEND_DEPTH = 2
            def emit_head_tail(h, p_avs, p_qs, denT16):
                tmp65 = invp.tile([65, 512], BF16, tag=f"tmp{h}",
                                  name=f"tmp{h}")
                # evacuation casts run on ScalarE: the DVE (custom-op exp) is
                # the fuller engine once EXP_G=512
                nc.scalar.copy(tmp65[:], p_avs[h][:])
                nc.sync.dma_start(outT[h * 64 : h * 64 + 64, p_qs],
                                  tmp65[0:64, :])
                nc.gpsimd.dma_start(dend[h : h + 1, :], tmp65[64:65, :])
                nc.gpsimd.dma_start(
                    denT16[:, 4 * h : 4 * h + 4],
                    dend[h : h + 1, :].rearrange("one (k p) -> p (one k)",
                                                 p=128),
                )

            for gs in range(NT + PEND_DEPTH):
                j, i = divmod(gs, NI)
                # ---- AV matmuls of step gs-2 FIRST: they are ready (their
                # exp finished mid-previous-step) and must not queue behind
                # this step's scores, which wait on exp(gs-2) via the psS
                # buffer rotation.
                if len(pendq) == PEND_DEPTH or (gs >= NT and pendq):
                    p_avs, p_est, p_est_v, p_start, p_stop, p_i, p_qs, p_j = \
                        pendq.pop(0)
                    va0 = vaug[:, p_i * VW + 0 * 65 : p_i * VW + 1 * 65]
                    va1 = vaug[:, p_i * VW + 1 * 65 : p_i * VW + 2 * 65]
                    nc.tensor.matmul(
                        p_avs[0][:], va0, p_est[:],
                        start=p_start, stop=p_stop,
                    )
                    nc.tensor.matmul(
                        p_avs[1][:], va1, p_est_v[:],
                        start=p_start, stop=p_stop,
                    )
                    if p_stop:
                        # psAV evacuation + den transposes: deferred to after
                        # this step's exp emission (same gs -- still ahead of
                        # the next block's first AV write at gs+1) so the DVE
                        # FIFO never front-runs exp with the 2x658ns casts.
                        denT16 = invp.tile([128, 8], BF16, tag="dT",
                                           name="dT")
                        st = {"j": p_j}
                        for h in range(2):
                            deferred.append(
                                (gs, lambda h=h, a=p_avs, q=p_qs, d=denT16:
                                 emit_head_tail(h, a, q, d))
                            )

                        def _fin(st=st, d=denT16):
                            st["invT"] = emit_recip(d)

                        deferred.append((gs + 8, _fin))
                        prev = st
                if gs < NT:
                    if i == 0:
                        avs = [
                            psAV.tile([65, 512], F32, tag=f"av{t}", name=f"av{t}")
                            for t in range(2)
                        ]
                    while stage_a_tasks and stage_a_tasks[0][0] <= gs + 3:
                        stage_a_tasks.pop(0)[1]()
                    if prev is not None and i >= 16 and i % 4 == 0:
                        fin2 = emit_proj(prev["j"], (i - 16) // 4,
                                         prev["invT"], after=last_sc)
                        deferred.append((gs + 1, fin2))
                    qs = bass.ts(j, 512)
                    ks = bass.ts(i, 128)
                    # per-head score tiles in per-head pools: head 0's exp
                    # runs on ScalarE, head 1's on the custom DVE op — each
                    # engine frees its own PSUM ring independently, and the
                    # est tiles stay fully decoupled (no WAW serialization).
                    pss0 = psS0.tile([128, 512], F32, tag="s0")
                    pss1 = psS1.tile([128, 512], F32, tag="s1")
                    nc.tensor.matmul(
                        pss0[:], kT[0:64, ks], qT[0:64, qs],
                        start=True, stop=True,
                    )
                    last_sc = nc.tensor.matmul(
                        pss1[:], kT[64:128, ks], qT[64:128, qs],
                        start=True, stop=True,
                    )
                    est_v = expp.tile([128, 512], BF16, tag="ev")
                    nc.vector._custom_dve(
                        EXP_OP,
                        out=est_v[:].bitcast(mybir.dt.uint16),
                        in0=pss1[:],
                        s0=EXP_A, s1=EXP_B, imm2=EXP_MAGIC,
                    )
                    est = expp.tile([128, 512], BF16, tag="e")
                    nc.scalar.activation(
                        est[:], pss0[:], AF.Exp, scale=SCALE,
                    )
                while deferred and (gs >= NT or deferred[0][0] <= gs):
                    deferred.pop(0)[1]()
                if gs < NT:
                    pendq.append((avs, est, est_v, i == 0, i == NI - 1, i,
                                  bass.ts(j, 512), j))
            # final block's tail (no next block hides it)
            lj = prev["j"]
            fins = [emit_proj(lj, k, prev["invT"]) for k in range(4)]
            for f in fins:
                f()

    nc.compile()
    return nc


def _pack_w(wslice):
    # [512, 128] -> SBUF image [128, 4*128] with C-chunk k at cols k*128..
    return np.ascontiguousarray(
        wslice.reshape(4, 128, 128).transpose(1, 0, 2).reshape(128, 512)
    ).astype(BFNP)


_NC_CACHE = None
LAST_RESULT = None


def kernel(x, W_qkv, W_proj, b_proj):
    global _NC_CACHE, LAST_RESULT
    x = np.asarray(x, dtype=np.float32)
    W_qkv = np.asarray(W_qkv, dtype=np.float32)
    W_proj = np.asarray(W_proj, dtype=np.float32)
    b_proj = np.asarray(b_proj, dtype=np.float32)

    if _NC_CACHE is None:
        _NC_CACHE = build_nc()
    nc = _NC_CACHE

    in_maps = []
    for c in range(N_CORES):
        b = c // 4
        h0 = 2 * (c % 4)
        xtb = np.ascontiguousarray(x[b].T).reshape(4, 128, N).astype(BFNP)
        wq = _pack_w(W_qkv[:, h0 * 64 : h0 * 64 + 128])
        wk = _pack_w(W_qkv[:, 512 + h0 * 64 : 512 + h0 * 64 + 128])
        wv = _pack_w(W_qkv[:, 1024 + h0 * 64 : 1024 + h0 * 64 + 128])
        wp = np.ascontiguousarray(
            W_proj[h0 * 64 : (h0 + 2) * 64, :]
        ).astype(BFNP)
        bias = (
            np.tile(b_proj[None, :], (128, 1)).astype(np.float32)
            if c % 4 == 0
            else np.zeros((128, 512), dtype=np.float32)
        )
        in_maps.append(
            {"xt": xtb, "wq": wq, "wk": wk, "wv": wv, "wp": wp, "bias": bias}
        )

    res = bass_utils.run_bass_kernel_spmd(
        nc, in_maps, core_ids=list(range(N_CORES))
    )
    LAST_RESULT = res

    out = np.zeros((B, N, C), dtype=np.float32)
    for c in range(N_CORES):
        out[c // 4] += res.results[c]["out"]
    return out



# revision 65
# speedup vs baseline: 1.1959x; 1.1956x over previous
"""Multi-head attention (B=2, N=4096, C=512, H=8, d=64) on 8 Trainium2 NeuronCores.

Sharding: core c handles batch b = c//4 and heads {2*(c%4), 2*(c%4)+1}.
Each core computes its 2 heads' attention plus a partial output projection
(contraction over its 128 rows of W_proj); the host gather sums the 4
partials per batch (bias is added on the p==0 core of each batch).

On-device dataflow (transposed-scores formulation, no on-chip transposes):
  qT/kT [128=2*64 d-dims, 4096]  = W.T @ x.T      (x.T supplied by host)
  v_aug [128 n-chunk, 32*(65+65)] = x @ Wv with a ones column per head
  S^T[kidx, q] = kT.T_chunk @ qT  (two heads ride row-groups 0-1 / 2-3
                                   of the PE array concurrently, K=64 each)
  E = exp(S^T / 8)                (ScalarE, scale folded into the LUT affine)
  [out_unnorm^T; den] = v_aug.T @ E   (ones column makes row 64 the softmax
                                       denominator -- no extra pass)
  out^T = out_unnorm^T * (1/den)  (reciprocal + K=1 broadcast matmul)
  partial = out^T.T @ W_proj_slice + bias   (per-head K=64 contractions)
"""

import sys
import types

for _p in ("/opt/trn_rl_repo",):
    if _p not in sys.path:
        sys.path.insert(0, _p)

import numpy as np
import ml_dtypes
from contextlib import ExitStack

# antenv.axon_hooks shim: lets run_bass_kernel_spmd find the NTFF profiling
# hook when BASS_TRACE=1 (the agent image's antenv lacks this module).
import antenv  # noqa: F401

if "antenv.axon_hooks" not in sys.modules:
    _m = types.ModuleType("antenv.axon_hooks")
    _m._hook = None

    def _set_hook(h):
        _m._hook = h

    def _get_hook():
        return _m._hook

    _m.set_axon_ntff_profile_hook = _set_hook
    _m.get_axon_ntff_profile_hook = _get_hook
    sys.modules["antenv.axon_hooks"] = _m
    try:
        from trn_agent_boot.trn_boot import _ntff_profile_via_ctypes

        hook = _ntff_profile_via_ctypes("/opt/axon/libaxon_pjrt.so")
        if hook is not None:
            _set_hook(hook)
    except Exception:
        pass

import concourse.bass as bass  # noqa: E402
import concourse.tile as tile  # noqa: E402
from concourse.tile import add_dep_helper  # noqa: E402
from concourse import mybir, bacc  # noqa: E402
from concourse import bass_utils  # noqa: E402
from concourse import dve_ops as _dvo  # noqa: E402
from concourse.dve_spec import (  # noqa: E402
    lower as _dve_lower, Spec as _Spec, Src0 as _Src0,
    C0 as _C0, C1 as _C1, C2 as _C2, _has_src1,
)
from concourse.dve_uop import DveOpSpec as _DveOpSpec  # noqa: E402

# No bucket storage in this container; artifacts stay local.
bass_utils.upload_artifacts = lambda tmpdir: f"local://{tmpdir}"

B, N, C = 2, 4096, 512
H, D = 8, 64
N_CORES = 8
SCALE = D ** -0.5

# DVE-offloaded exp (Schraudolph in bf16-bit domain): a custom 3-stage
# DVE op computes round(s*EXP_A + EXP_B) via the fp32 magic-add trick
# ((x*A + B+M) - M forces round-to-integer at the +M add); the integral
# fp32 result is the bf16 BIT PATTERN of exp(s*SCALE)*(1+eps),
# |eps| <~ 4% mean-zero (cancels in the softmax ratio), and is written
# through a uint16-bitcast AP straight into the bf16 est tile.
EXP_G = 512           # q-columns per step exp'd on DVE instead of ScalarE
EXP_A = SCALE * 1.4426950408889634 * 128.0
EXP_SIGMA = 0.056
EXP_MAGIC = 12582912.0  # 1.5 * 2^23
EXP_B = 128.0 * (127.0 - EXP_SIGMA) + EXP_MAGIC

_exp_spec = _Spec(
    body=(_Src0 * _C0 + _C1) - _C2,
    reference=lambda in0, in1, s0, s1, imm2: (
        (in0.astype(np.float32) * np.float32(s0) + np.float32(s1)).astype(
            np.float32
        )
        - np.float32(imm2)
    ),
)
if "EXP_BF16_BITS_ANT" not in _dvo._SUB_OPCODE_FOR_NAME:
    _row = _dvo._CUSTOM_DVE_ROW_BASE + len(_dvo.OPS)
    _shas = {}
    for _ver in ("v3", "v4"):
        _s = _DveOpSpec(
            name="EXP_BF16_BITS_ANT", opcode=_row,
            uops=_dve_lower(_exp_spec, ver=_ver), rd1_en=_has_src1(_exp_spec),
        )
        _shas[_ver] = _s.sha(_ver)
    EXP_OP = _dvo.DveOp(
        "EXP_BF16_BITS_ANT", _exp_spec, subdim=False, uops_sha=_shas
    )
    _dvo.OPS.append(EXP_OP)
    _dvo.CUSTOM_DVE_SPECS[EXP_OP.name] = _exp_spec
    _dvo._SUB_OPCODE_FOR_NAME[EXP_OP.name] = _row
else:
    EXP_OP = next(o for o in _dvo.OPS if o.name == "EXP_BF16_BITS_ANT")

BF16 = mybir.dt.bfloat16
F32 = mybir.dt.float32
AF = mybir.ActivationFunctionType
BFNP = ml_dtypes.bfloat16

NI = N // 128   # 32 kidx / n chunks
NJ = N // 1024  # 4 q blocks
VW = 2 * (D + 1)  # 130: per-n-chunk vaug block (2 heads x (64 v + 1 ones))


def build_nc():
    nc = bacc.Bacc("TRN2", target_bir_lowering=False, debug=False)

    xt = nc.dram_tensor("xt", [4, 128, N], BF16, kind="ExternalInput").ap()
    wq = nc.dram_tensor("wq", [128, 512], BF16, kind="ExternalInput").ap()
    wk = nc.dram_tensor("wk", [128, 512], BF16, kind="ExternalInput").ap()
    wv = nc.dram_tensor("wv", [128, 512], BF16, kind="ExternalInput").ap()
    # wp rows 0-63 = W_proj rows of head 0, rows 64-127 = head 1 (the two
    # projection contractions ride PE row-groups 0-1 / 2-3 concurrently)
    wp = nc.dram_tensor("wp", [128, 512], BF16, kind="ExternalInput").ap()
    # bias pre-replicated across partitions; fp32 (added via affine_then_add)
    bias = nc.dram_tensor("bias", [128, 512], F32, kind="ExternalInput").ap()
    out = nc.dram_tensor("out", [N, C], F32, kind="ExternalOutput").ap()
    # DRAM bounce buffer for transposing den rows to partition-major
    # (SBUF source APs cannot step partitions by elements; DRAM is linear)
    dend = nc.dram_tensor("dend", [2, 512], BF16, kind="Internal").ap()

    with tile.TileContext(nc) as tc:
        with ExitStack() as ctx:
            const = ctx.enter_context(tc.tile_pool(name="const", bufs=1))
            sb = ctx.enter_context(tc.tile_pool(name="sb", bufs=1))
            expp = ctx.enter_context(tc.tile_pool(name="expp", bufs=4))
            invp = ctx.enter_context(tc.tile_pool(name="invp", bufs=2))
            outp = ctx.enter_context(tc.tile_pool(name="outp", bufs=3))

            wrmw = const.tile([128, 128], BF16)
            nc.gpsimd.memset(wrmw[:], 0.5)
            twq = const.tile([128, 512], BF16)
            nc.gpsimd.dma_start(twq[:], wq[:])
            twk = const.tile([128, 512], BF16)
            nc.gpsimd.dma_start(twk[:], wk[:])
            twv = const.tile([128, 512], BF16)
            nc.gpsimd.dma_start(twv[:], wv[:])
            # twp/biasr are not needed until the first block tail (~step 48);
            # their DMAs are emitted right before the main loop so they don't
            # compete with the x.T load on the critical path to score 0.
            twp = const.tile([128, 512], BF16)
            biasr = const.tile([128, 512], F32)

            qT = sb.tile([128, N], BF16)
            kT = sb.tile([128, N], BF16)
            vaug = sb.tile([128, NI * VW], BF16)
            nc.gpsimd.memset(vaug[:], 1.0)
            # unnormalized out^T, both heads stacked: h0 rows 0-63, h1 rows
            # 64-127 (normalization by 1/den is folded into the projection
            # evacuation via per-partition affine_then_add scales)
            outT = sb.tile([128, N], BF16)

            xtp = ctx.enter_context(tc.tile_pool(name="xtp", bufs=1))
            psS0 = ctx.enter_context(tc.tile_pool(name="psS0", bufs=2, space="PSUM"))
            psS1 = ctx.enter_context(tc.tile_pool(name="psS1", bufs=2, space="PSUM"))
            psAV = ctx.enter_context(tc.tile_pool(name="psAV", bufs=1, space="PSUM"))
            psT = ctx.enter_context(tc.tile_pool(name="psT", bufs=2, space="PSUM"))

            # ---- stage A: QKV projections ------------------------------
            # Emitted as deadline-scheduled tasks threaded into the first two
            # j-blocks' i-loops (the PE queue is strict FIFO; anything emitted
            # before the first score matmul delays the first exp).
            xts = []
            for k in range(4):
                t = xtp.tile([128, N], BF16, tag=f"xt{k}", name=f"xt{k}")
                xts.append(t)
            for col in range(8):
                for k in range(4):
                    cs = bass.ts(col, N // 8)
                    nc.sync.dma_start(xts[k][:, cs], xt[k][:, cs])

            # PE warmup: the HAM clock gate releases (1.2 -> 2.4 GHz) only
            # after ~3.4us of sustained PE activity; burn that window on
            # dummy matmuls (on a memset tile, so they start immediately)
            # while the x.T DMA lands, so stage A runs warm.
            warm = psT.tile([128, 64], F32, tag="t", name="warm")
            for _ in range(48):
                nc.tensor.matmul(
                    warm[:], wrmw[:, 0:128], wrmw[:, 0:64],
                    start=True, stop=True,
                )

            def emit_qk(j8, which):
                s_ = bass.ts(j8, 512)
                w, dst = (twq, qT) if which == "q" else (twk, kT)
                ps = psT.tile([128, 512], F32, tag="t", name="psqk")
                for k in range(4):
                    nc.tensor.matmul(
                        ps[:], w[:, bass.ts(k, 128)], xts[k][:, s_],
                        start=(k == 0), stop=(k == 3),
                    )
                nc.vector.tensor_copy(dst[:, s_], ps[:])

            def emit_v(jj):
                ps = psT.tile([128, 128], F32, tag="t", name="psv")
                for k in range(4):
                    nc.tensor.matmul(
                        ps[:], xts[k][:, bass.ts(jj, 128)], twv[:, bass.ts(k, 128)],
                        start=(k == 0), stop=(k == 3),
                    )
                dst = vaug[:, jj * VW : (jj + 1) * VW].rearrange(
                    "p (h c) -> p h c", h=2
                )[:, :, 0:D]
                src = ps[:].rearrange("p (h c) -> p h c", h=2)
                nc.vector.tensor_copy(dst, src)

            # (deadline in global i-steps, emitter) — qk k-chunk c feeds
            # scores at step 4c; v chunk jj feeds the AV matmul at step jj;
            # qk q-chunk j8 feeds block j8 (step 32*j8).
            stage_a_tasks = []
            for c in range(1, 8):
                stage_a_tasks.append((4 * c - 4, lambda c=c: emit_qk(c, "k")))
            for jj in range(1, NI):
                stage_a_tasks.append((jj - 2, lambda jj=jj: emit_v(jj)))
            for j8 in range(1, 8):
                stage_a_tasks.append((32 * j8 - 6, lambda j8=j8: emit_qk(j8, "q")))
            stage_a_tasks.sort(key=lambda t: t[0])
            stage_a_tasks = list(stage_a_tasks)

            # prologue: what step 0 needs
            emit_qk(0, "q")
            emit_qk(0, "k")
            emit_v(0)
            nc.gpsimd.dma_start(twp[:], wp[:])
            nc.gpsimd.dma_start(biasr[:], bias[:])

            # ---- stage B: scores^T -> exp -> AV (+den) ------------------
            # ---- stage C: paired projection, normalize folded in --------
            # Block tail: evacuate psAV to bf16 outT (h1 hops to partitions
            # 64-127 via SBUF DMA), transpose the den rows to a [128, 8]
            # partition-major tile via the DMA xbar, one fast reciprocal,
            # then per-jj: two row-group-CONCURRENT projection matmuls into
            # separate PSUM tiles and two affine_then_add evacuations that
            # apply 1/den (per-partition scale) and the bias.
            def emit_recip(denT16):
                denT32 = invp.tile([128, 8], F32, tag="d32", name="d32")
                nc.vector.tensor_copy(denT32[:], denT16[:])
                invT = invp.tile([128, 8], F32, tag="invT", name="invT")
                nc.vector.reciprocal_approx_fast(invT[:], denT32[:])
                return invT

            def emit_proj(j, k, invT, after=None):
                # Returns a closure finishing the second half (ATA #2 + store)
                # so the caller can defer it one step: two back-to-back 690ns
                # DVE ATAs would head-of-line-block the est32 op the next AV
                # matmul needs.
                jj = j * 4 + k
                s = bass.ts(jj, 128)
                pp0 = psT.tile([128, 512], F32, tag="t", name="pp0")
                mi = nc.tensor.matmul(
                    pp0[:], outT[0:64, s], twp[0:64, :], start=True, stop=True
                )
                if after is not None:
                    add_dep_helper(mi.ins, after.ins, sync=False,
                                   reason="tail behind scores")
                pp1 = psT.tile([128, 512], F32, tag="t", name="pp1")
                nc.tensor.matmul(
                    pp1[:], outT[64:128, s], twp[64:128, :],
                    start=True, stop=True,
                )
                ota = outp.tile([128, 512], F32, tag="oa", name="ota")
                nc.vector.affine_then_add(
                    ota[:], pp0[:], biasr[:], invT[:, k : k + 1], 0.0
                )

                def fin2():
                    ot = outp.tile([128, 512], F32, tag="o", name="ot")
                    nc.vector.affine_then_add(
                        ot[:], pp1[:], ota[:], invT[:, 4 + k : 4 + k + 1], 0.0
                    )
                    nc.sync.dma_start(out[s, :], ot[:])

                return fin2

            # Flat software pipeline over all 256 i-steps. AV matmuls are
            # emitted one step behind their scores/exp so the PE queue always
            # holds independent score work when an AV has to wait (block
            # boundary: the new accumulator bank frees only after the old
            # one's DVE evacuation).
            prev = None   # pending proj tail of the finished block
            # AV emission delayed TWO steps behind its scores: the exp of a
            # step is serialized (ScalarE half then DVE half -- same est tile,
            # so the framework orders the writers) at ~1.35us latency; depth-2
            # pipelining keeps that latency entirely off the PE critical path.
            pendq = []
            avs = None
            deferred = []  # per-block tail pieces drained one per step
            NT = 8 * NI
            PEND_DEPTH = 2
            def emit_head_tail(h, p_avs, p_qs, denT16):
                tmp65 = invp.tile([65, 512], BF16, tag=f"tmp{h}",
                                  name=f"tmp{h}")
                # split the two evacuation casts across ScalarE and DVE so
                # neither engine's FIFO takes the whole 1.3us tail burst
                if h == 0:
                    nc.scalar.copy(tmp65[:], p_avs[h][:])
                else:
                    nc.vector.tensor_copy(tmp65[:], p_avs[h][:])
                nc.sync.dma_start(outT[h * 64 : h * 64 + 64, p_qs],
                                  tmp65[0:64, :])
                nc.gpsimd.dma_start(dend[h : h + 1, :], tmp65[64:65, :])
                nc.gpsimd.dma_start(
                    denT16[:, 4 * h : 4 * h + 4],
                    dend[h : h + 1, :].rearrange("one (k p) -> p (one k)",
                                                 p=128),
                )

            for gs in range(NT + PEND_DEPTH):
                j, i = divmod(gs, NI)
                # ---- AV matmuls of step gs-2 FIRST: they are ready (their
                # exp finished mid-previous-step) and must not queue behind
                # this step's scores, which wait on exp(gs-2) via the psS
                # buffer rotation.
                if len(pendq) == PEND_DEPTH or (gs >= NT and pendq):
                    p_avs, p_est, p_est_v, p_start, p_stop, p_i, p_qs, p_j = \
                        pendq.pop(0)
                    va0 = vaug[:, p_i * VW + 0 * 65 : p_i * VW + 1 * 65]
                    va1 = vaug[:, p_i * VW + 1 * 65 : p_i * VW + 2 * 65]
                    nc.tensor.matmul(
                        p_avs[0][:], va0, p_est[:],
                        start=p_start, stop=p_stop,
                    )
                    nc.tensor.matmul(
                        p_avs[1][:], va1, p_est_v[:],
                        start=p_start, stop=p_stop,
                    )
                    if p_stop:
                        # psAV evacuation + den transposes: deferred to after
                        # this step's exp emission (same gs -- still ahead of
                        # the next block's first AV write at gs+1) so the DVE
                        # FIFO never front-runs exp with the 2x658ns casts.
                        denT16 = invp.tile([128, 8], BF16, tag="dT",
                                           name="dT")
                        st = {"j": p_j}
                        for h in range(2):
                            deferred.append(
                                (gs, lambda h=h, a=p_avs, q=p_qs, d=denT16:
                                 emit_head_tail(h, a, q, d))
                            )

                        def _fin(st=st, d=denT16):
                            st["invT"] = emit_recip(d)

                        deferred.append((gs + 8, _fin))
                        prev = st
                if gs < NT:
                    if i == 0:
                        avs = [
                            psAV.tile([65, 512], F32, tag=f"av{t}", name=f"av{t}")
                            for t in range(2)
                        ]
                    while stage_a_tasks and stage_a_tasks[0][0] <= gs + 3:
                        stage_a_tasks.pop(0)[1]()
                    if prev is not None and i >= 16 and i % 4 == 0:
                        fin2 = emit_proj(prev["j"], (i - 16) // 4,
                                         prev["invT"], after=last_sc)
                        deferred.append((gs + 1, fin2))
                    qs = bass.ts(j, 512)
                    ks = bass.ts(i, 128)
                    # per-head score tiles in per-head pools: head 0's exp
                    # runs on ScalarE, head 1's on the custom DVE op — each
                    # engine frees its own PSUM ring independently, and the
                    # est tiles stay fully decoupled (no WAW serialization).
                    pss0 = psS0.tile([128, 512], F32, tag="s0")
                    pss1 = psS1.tile([128, 512], F32, tag="s1")
                    nc.tensor.matmul(
                        pss0[:], kT[0:64, ks], qT[0:64, qs],
                        start=True, stop=True,
                    )
                    last_sc = nc.tensor.matmul(
                        pss1[:], kT[64:128, ks], qT[64:128, qs],
                        start=True, stop=True,
                    )
                    est_v = expp.tile([128, 512], BF16, tag="ev")
                    nc.vector._custom_dve(
                        EXP_OP,
                        out=est_v[:].bitcast(mybir.dt.uint16),
                        in0=pss1[:],
                        s0=EXP_A, s1=EXP_B, imm2=EXP_MAGIC,
                    )
                    est = expp.tile([128, 512], BF16, tag="e")
                    nc.scalar.activation(
                        est[:], pss0[:], AF.Exp, scale=SCALE,
                    )
                while deferred and (gs >= NT or deferred[0][0] <= gs):
                    deferred.pop(0)[1]()
                if gs < NT:
                    pendq.append((avs, est, est_v, i == 0, i == NI - 1, i,
                                  bass.ts(j, 512), j))
            # final block's tail (no next block hides it)
            lj = prev["j"]
            fins = [emit_proj(lj, k, prev["invT"]) for k in range(4)]
            for f in fins:
                f()

    nc.compile()
    return nc


def _pack_w(wslice):
    # [512, 128] -> SBUF image [128, 4*128] with C-chunk k at cols k*128..
    return np.ascontiguousarray(
        wslice.reshape(4, 128, 128).transpose(1, 0, 2).reshape(128, 512)
    ).astype(BFNP)


_NC_CACHE = None
LAST_RESULT = None


def kernel(x, W_qkv, W_proj, b_proj):
    global _NC_CACHE, LAST_RESULT
    x = np.asarray(x, dtype=np.float32)
    W_qkv = np.asarray(W_qkv, dtype=np.float32)
    W_proj = np.asarray(W_proj, dtype=np.float32)
    b_proj = np.asarray(b_proj, dtype=np.float32)

    if _NC_CACHE is None:
        _NC_CACHE = build_nc()
    nc = _NC_CACHE

    in_maps = []
    for c in range(N_CORES):
        b = c // 4
        h0 = 2 * (c % 4)
        xtb = np.ascontiguousarray(x[b].T).reshape(4, 128, N).astype(BFNP)
        wq = _pack_w(W_qkv[:, h0 * 64 : h0 * 64 + 128])
        wk = _pack_w(W_qkv[:, 512 + h0 * 64 : 512 + h0 * 64 + 128])
        wv = _pack_w(W_qkv[:, 1024 + h0 * 64 : 1024 + h0 * 64 + 128])
        wp = np.ascontiguousarray(
            W_proj[h0 * 64 : (h0 + 2) * 64, :]
        ).astype(BFNP)
        bias = (
            np.tile(b_proj[None, :], (128, 1)).astype(np.float32)
            if c % 4 == 0
            else np.zeros((128, 512), dtype=np.float32)
        )
        in_maps.append(
            {"xt": xtb, "wq": wq, "wk": wk, "wv": wv, "wp": wp, "bias": bias}
        )

    res = bass_utils.run_bass_kernel_spmd(
        nc, in_maps, core_ids=list(range(N_CORES))
    )
    LAST_RESULT = res

    out = np.zeros((B, N, C), dtype=np.float32)
    for c in range(N_CORES):
        out[c // 4] += res.results[c]["out"]
    return out

